# revision 1
# baseline (speedup 1.0000x reference)
"""Trainium2 Bass kernel for nn_CorrTorch_unfold (B=1, C=32, D=32, H=W=128).

Math (flat-remap unfold, see reference.py): per depth slice d
  out[k2, h2, w2] = lrelu( sum_c x[c,d,h2,w2] * y_pad[c', d, h'+kh, w'+kw+c] )
  with n = 9c'+k' (k'=(kh,kw)), (k2, m) = divmod(n, 32),
  h2 = 4m+t, w2 = 4wb+j, partition p = h' = 32t+wb.

On-chip layout per slice (all fp16):
  A[p, kh*4160 + c'*130 + w] = y_pad[c', d, p+kh, w]     (3 row-shifted loads)
  XT[p, s*128 + j*32 + c]    = x[c, d, 4*(s%32)+t, 4wb+j] (host-packed, 40
                                m-slots: s=32..39 duplicate m=0..7 so the
                                (9c')%32 + 3kh + kw slot index never wraps)
  per c'-unit: products P[p, kh*384+kw*128+j*32+c] = A-slice * XT-slice
  (one [128,1152] tensor_tensor, 2x fp16 mode), then a shrinking TT-add
  tree (16+8+4+2+1) sums c-groups of 32 -> OS[p, c'*36 + g].
  lrelu = 0.6*OS + 0.4*|OS| via two ACT passes + one TT add.

Engines: DVE does most units; GpSimd (Pool) takes POOL_UNITS c'-units per
slice (its own mults + batched tree). ACT does the lrelu helper passes.

Sharding: D=32 depth slices, 4 per core across 8 cores. Host packs/unpacks
(pure permutations); device output is OS-packed [d, 128, 1152] fp16.
"""
import numpy as np

_PROG_CACHE = {}
_RUN_OPTS = {"trace": False}
_LAST_RESULT = {}

D_LOC = 4
N_CORES = 8
C = 32
H = W = 128
WPAD = 130
ABLK = C * WPAD          # 4160 elems per kh block of A
NSLOT = 40               # XT m-slots (8 duplicated)
POOL_UNITS = 8           # c'-units per slice run on GpSimd (c' 32-POOL..31)


def _build_program():
    import concourse.bass as bass
    import concourse.bacc as bacc
    import concourse.mybir as mybir
    from concourse.tile import TileContext
    from bass_rust import VecI64Pair

    f16 = mybir.dt.float16
    f32 = mybir.dt.float32
    mult = mybir.AluOpType.mult
    add = mybir.AluOpType.add

    def apv(base_ap, offset, dims):
        a = base_ap.copy()
        part = list(a.ap[0])
        a.ap = VecI64Pair([part] + [list(d) for d in dims])
        a.offset = a.offset + offset
        return a

    nc = bacc.Bacc()
    # per-core inputs: xt slab (host-packed XT), y slab pre-shifted+padded
    xt_in = nc.dram_tensor("xtin", [D_LOC, 128, NSLOT * 128], f16,
                           kind="ExternalInput")
    y_in = nc.dram_tensor("yin", [D_LOC, C, WPAD, WPAD], f16,
                          kind="ExternalInput")
    out = nc.dram_tensor("out", [D_LOC, 128, 1152], f16,
                         kind="ExternalOutput")

    DVE_UNITS = 32 - POOL_UNITS

    with TileContext(nc) as tc:
        with tc.tile_pool(name="a", bufs=2) as apool, \
             tc.tile_pool(name="xt", bufs=2) as xtpool, \
             tc.tile_pool(name="pr", bufs=1) as prpool, \
             tc.tile_pool(name="tr", bufs=1) as trpool, \
             tc.tile_pool(name="os", bufs=2) as ospool, \
             tc.tile_pool(name="ab", bufs=1) as abpool, \
             tc.tile_pool(name="ot", bufs=2) as otpool:

            for d in range(D_LOC):
                # ---- loads ----
                A = apool.tile([128, 3 * ABLK], f16)
                for kh in range(3):
                    src = y_in[:].copy()
                    src.ap = VecI64Pair(
                        [[WPAD, 128], [WPAD * WPAD, C], [1, WPAD]])
                    src.offset = d * C * WPAD * WPAD + kh * WPAD
                    dst = A[:, kh * ABLK:(kh + 1) * ABLK].rearrange(
                        "p (c w) -> p c w", c=C)
                    nc.sync.dma_start(dst, src)

                XT = xtpool.tile([128, NSLOT * 128], f16)
                nc.sync.dma_start(XT[:], xt_in[d])

                OS = ospool.tile([128, 1152], f16)

                # ---- per-chunk mults + tree (DVE: c' 0..23, Pool: 24..31)
                def do_chunk(eng, c0, c1, tag):
                    ncp = c1 - c0
                    P = prpool.tile([128, ncp * 1152], f16, tag=f"p{tag}")
                    for i, cp in enumerate(range(c0, c1)):
                        m0 = (9 * cp) % 32
                        in0 = apv(A[:], cp * WPAD,
                                  [[ABLK, 3], [1, 3], [1, 128]])
                        in1 = apv(XT[:], m0 * 128,
                                  [[384, 3], [128, 3], [1, 128]])
                        po = P[:, i * 1152:(i + 1) * 1152].rearrange(
                            "p (a b f) -> p a b f", a=3, b=3)
                        eng.tensor_tensor(po, in0, in1, mult)
                    # tree: 36*ncp groups of 32 -> sums
                    g = 36 * ncp
                    T1 = trpool.tile([128, g * 16], f16, tag=f"t1{tag}")
                    eng.tensor_tensor(
                        T1[:].rearrange("p (g c) -> p g c", c=16),
                        apv(P[:], 0, [[32, g], [1, 16]]),
                        apv(P[:], 16, [[32, g], [1, 16]]), add)
                    T2 = trpool.tile([128, g * 8], f16, tag=f"t2{tag}")
                    eng.tensor_tensor(
                        T2[:].rearrange("p (g c) -> p g c", c=8),
                        apv(T1[:], 0, [[16, g], [1, 8]]),
                        apv(T1[:], 8, [[16, g], [1, 8]]), add)
                    T3 = trpool.tile([128, g * 4], f16, tag=f"t3{tag}")
                    eng.tensor_tensor(
                        T3[:].rearrange("p (g c) -> p g c", c=4),
                        apv(T2[:], 0, [[8, g], [1, 4]]),
                        apv(T2[:], 4, [[8, g], [1, 4]]), add)
                    T4 = trpool.tile([128, g * 2], f16, tag=f"t4{tag}")
                    eng.tensor_tensor(
                        T4[:].rearrange("p (g c) -> p g c", c=2),
                        apv(T3[:], 0, [[4, g], [1, 2]]),
                        apv(T3[:], 2, [[4, g], [1, 2]]), add)
                    eng.tensor_tensor(
                        OS[:, c0 * 36:c1 * 36],
                        apv(T4[:], 0, [[2, g]]),
                        apv(T4[:], 1, [[2, g]]), add)

                for ch in range(3):
                    do_chunk(nc.vector, 8 * ch, 8 * ch + 8, "v")
                if POOL_UNITS:
                    do_chunk(nc.gpsimd, 32 - POOL_UNITS, 32, "g")

                # ---- leaky relu: out = 0.6*OS + 0.4*|OS| ----
                AB = abpool.tile([128, 1152], f16, tag="ab")
                CC = abpool.tile([128, 1152], f16, tag="cc")
                nc.scalar.activation(AB[:], OS[:],
                                     mybir.ActivationFunctionType.Abs,
                                     scale=0.4)
                nc.scalar.activation(CC[:], OS[:],
                                     mybir.ActivationFunctionType.Copy,
                                     scale=0.6)
                OT = otpool.tile([128, 1152], f16, tag="ot")
                nc.vector.tensor_tensor(OT[:], CC[:], AB[:], add)
                nc.sync.dma_start(out[d], OT[:])

    nc.finalize()
    return nc


def _get_program():
    if "nc" not in _PROG_CACHE:
        _PROG_CACHE["nc"] = _build_program()
    return _PROG_CACHE["nc"]


def _pack_xt(x):  # x [1,32,32,128,128] f32 -> [32, 128, 5120] fp16
    B, C_, D, H_, W_ = x.shape
    xt = np.zeros((D, 128, NSLOT, 128), np.float32)
    xd = x[0]  # [C, D, H, W]
    s = np.arange(NSLOT)
    m = s % 32
    for t in range(4):
        v = xd[:, :, 4 * m + t, :].reshape(C_, D, NSLOT, 32, 4)  # c d s wb j
        xt[:, 32 * t:32 * t + 32, :, :] = (
            v.transpose(1, 3, 2, 4, 0).reshape(D, 32, NSLOT, 128))
    return np.ascontiguousarray(xt.reshape(D, 128, NSLOT * 128)
                                ).astype(np.float16)


def kernel(x: np.ndarray, y: np.ndarray) -> np.ndarray:
    from concourse.bass_utils import run_bass_kernel_spmd

    x = np.ascontiguousarray(np.asarray(x, dtype=np.float32))
    y = np.ascontiguousarray(np.asarray(y, dtype=np.float32))
    B, C_, D, H_, W_ = x.shape
    assert (B, C_, D, H_, W_) == (1, 32, 32, 128, 128)

    # host prep: depth-shifted, H/W-padded y (fp16); packed XT slabs
    y_sp = np.zeros((D, C_, WPAD, WPAD), np.float16)
    y_sp[1:, :, 1:129, 1:129] = y[0].transpose(1, 0, 2, 3)[:-1].astype(
        np.float16)
    xt = _pack_xt(x)

    nc = _get_program()
    in_maps = [
        {"xtin": xt[4 * j:4 * j + 4], "yin": y_sp[4 * j:4 * j + 4]}
        for j in range(N_CORES)
    ]
    res = run_bass_kernel_spmd(nc, in_maps, core_ids=list(range(N_CORES)),
                               trace=_RUN_OPTS["trace"])
    _LAST_RESULT["res"] = res
    packed = np.concatenate(
        [np.asarray(res.results[j]["out"], np.float32)
         for j in range(N_CORES)], axis=0)  # [32, 128, 1152]

    # host unpermute: [d, p, c'*36 + k*4 + j] -> [1, 9, D, H, W]
    a = packed.reshape(D, 4, 32, 32, 9, 4)                 # d t wb c' k j
    a = a.transpose(3, 4, 0, 1, 2, 5)                      # c' k d t wb j
    a = np.ascontiguousarray(a).reshape(9, 32, D, 4, 32, 4)  # k2 m d t wb j
    a = a.transpose(0, 2, 1, 3, 4, 5)                      # k2 d m t wb j
    a = np.ascontiguousarray(a).reshape(9, D, 128, 128)
    return a[None].astype(np.float32)



# revision 7
# speedup vs baseline: 1.1987x; 1.1987x over previous
"""Trainium2 Bass kernel for nn_CorrTorch_unfold (B=1, C=32, D=32, H=W=128).

Math (flat-remap unfold, see reference docstring): per depth slice d
  out[k2, h2, w2] = lrelu( sum_c x[c,d,h2,w2] * y_pad[c', d, h'+kh, w'+kw+c] )
  with n = 9c'+k' (k'=(kh,kw)), (k2, m) = divmod(n, 32),
  h2 = 4m+t, w2 = 4wb+j, partition p = h' = 32t+wb.

v2 design (all fp16, everything on DVE at the 2x_1p rate):
  - GpSimd (Pool) is NOT used: concurrent Pool tensor_tensor degrades DVE
    ~2-4x via the shared SBUF port pair (measured), a large net loss.
  - A[p, kh*4160 + c'*130 + w] = y_pad[c', d, p+kh, w]   (3 row-shifted DMAs)
  - XT64[p, s*128 + j*32 + c] = x[c, d, 4*(s%32)+t, 4wb+j], 64 m-slots
    (two copies of the 32 m-slots) so each 4-unit product group reads a
    fully contiguous 4608-elem slab: group k covers slots 4k .. 4k+35.
  - products: per group of 4 c'-units one TT mult [128, 4608]
    (in0 A 4-dim strided AP, in1/out contiguous, all even offsets -> 2x).
  - tree: chunks of 16 units (g=576), strided pairwise adds, all even
    offsets (2x); last level via "plane-split" T4 (1x write) so the final
    add reads two contiguous planes instead of an odd-offset stride-2 AP.
  - lrelu = 0.6*OS + 0.4*|OS|: two ACT passes + one DVE add (ACT has its
    own SBUF ports; never contends).

Sharding: D=32 depth slices, 4 per core across 8 cores. Host packs/unpacks
(pure permutations); device output is OS-packed [d, 128, 1152] fp16.
"""
import numpy as np

_PROG_CACHE = {}
_RUN_OPTS = {"trace": False}
_LAST_RESULT = {}

D_LOC = 4
N_CORES = 8
C = 32
H = W = 128
WPAD = 130
ABLK = C * WPAD          # 4160 elems per kh block of A
NSLOT = 64               # XT m-slots (two copies of 32)


def _build_program():
    import concourse.bass as bass
    import concourse.bacc as bacc
    import concourse.mybir as mybir
    from concourse.tile import TileContext
    from bass_rust import VecI64Pair

    f16 = mybir.dt.float16
    mult = mybir.AluOpType.mult
    add = mybir.AluOpType.add

    def apv(base_ap, offset, dims):
        a = base_ap.copy()
        part = list(a.ap[0])
        a.ap = VecI64Pair([part] + [list(d) for d in dims])
        a.offset = a.offset + offset
        return a

    nc = bacc.Bacc()
    xt_in = nc.dram_tensor("xtin", [D_LOC, 128, NSLOT * 128], f16,
                           kind="ExternalInput")
    y_in = nc.dram_tensor("yin", [D_LOC, C, WPAD, WPAD], f16,
                          kind="ExternalInput")
    out = nc.dram_tensor("out", [D_LOC, 128, 1152], f16,
                         kind="ExternalOutput")

    with TileContext(nc) as tc:
        with tc.tile_pool(name="a", bufs=2) as apool, \
             tc.tile_pool(name="xt", bufs=2) as xtpool, \
             tc.tile_pool(name="pr", bufs=1) as prpool, \
             tc.tile_pool(name="tr", bufs=1) as trpool, \
             tc.tile_pool(name="os", bufs=2) as ospool, \
             tc.tile_pool(name="ab", bufs=1) as abpool, \
             tc.tile_pool(name="ot", bufs=2) as otpool:

            for d in range(D_LOC):
                # ---- loads ----
                A = apool.tile([128, 3 * ABLK], f16)
                for kh in range(3):
                    src = y_in[:].copy()
                    src.ap = VecI64Pair(
                        [[WPAD, 128], [WPAD * WPAD, C], [1, WPAD]])
                    src.offset = d * C * WPAD * WPAD + kh * WPAD
                    dst = A[:, kh * ABLK:(kh + 1) * ABLK].rearrange(
                        "p (c w) -> p c w", c=C)
                    nc.sync.dma_start(dst, src)

                XT = xtpool.tile([128, NSLOT * 128], f16)
                nc.sync.dma_start(XT[:], xt_in[d])

                OS = ospool.tile([128, 1152], f16)

                # ---- 2 chunks of 16 c'-units ----
                for ch in range(2):
                    c0 = 16 * ch
                    P = prpool.tile([128, 16 * 1152], f16, tag="p")
                    # products: 4 groups of 4 units x 3 kh, one TT each
                    # (TensorTensor ISA mem patterns allow at most 3 free
                    # dims, so the kh axis gets its own instruction)
                    for gi in range(4):
                        kk = 4 * ch + gi            # global group id 0..7
                        cp0 = 4 * kk                # first c' of group
                        s0 = 4 * kk                 # first XT slot of group
                        for kh in range(3):
                            in0 = apv(A[:], kh * ABLK + cp0 * WPAD,
                                      [[WPAD, 4], [1, 3], [1, 128]])
                            in1 = apv(XT[:], (s0 + 3 * kh) * 128,
                                      [[1152, 4], [1, 384]])
                            po = apv(P[:], gi * 4608 + kh * 384,
                                     [[1152, 4], [1, 384]])
                            nc.vector.tensor_tensor(po, in0, in1, mult)
                    # tree over c2 (innermost 32), g = 16*36 = 576 groups
                    g = 576
                    T1 = trpool.tile([128, g * 16], f16, tag="t1")
                    nc.vector.tensor_tensor(
                        apv(T1[:], 0, [[1, g * 16]]),
                        apv(P[:], 0, [[32, g], [1, 16]]),
                        apv(P[:], 16, [[32, g], [1, 16]]), add)
                    T2 = trpool.tile([128, g * 8], f16, tag="t2")
                    nc.vector.tensor_tensor(
                        apv(T2[:], 0, [[1, g * 8]]),
                        apv(T1[:], 0, [[16, g], [1, 8]]),
                        apv(T1[:], 8, [[16, g], [1, 8]]), add)
                    # T3 sums c2-bit2.  4-elem-run reads measured ~0.44x,
                    # so split into two 2-elem-run instructions, each
                    # writing a contiguous (g, b0) plane for its b1 value.
                    # T3 layout: quad q holds [b1=0 pair | b1=1 pair] so
                    # T4 reads the proven-fast [[4,g],[1,2]] pattern.
                    T3 = trpool.tile([128, g * 4], f16, tag="t3")
                    nc.vector.tensor_tensor(
                        apv(T3[:], 0, [[4, g], [1, 2]]),
                        apv(T2[:], 0, [[8, g], [1, 2]]),
                        apv(T2[:], 4, [[8, g], [1, 2]]), add)
                    nc.vector.tensor_tensor(
                        apv(T3[:], 2, [[4, g], [1, 2]]),
                        apv(T2[:], 2, [[8, g], [1, 2]]),
                        apv(T2[:], 6, [[8, g], [1, 2]]), add)
                    # T4: output plane-split by b0 so T5 reads two
                    # contiguous planes.
                    T4 = trpool.tile([128, g * 2], f16, tag="t4")
                    nc.vector.tensor_tensor(
                        apv(T4[:], 0, [[1, g], [g, 2]]),
                        apv(T3[:], 0, [[4, g], [1, 2]]),
                        apv(T3[:], 2, [[4, g], [1, 2]]), add)
                    nc.vector.tensor_tensor(
                        apv(OS[:], c0 * 36, [[1, g]]),
                        apv(T4[:], 0, [[1, g]]),
                        apv(T4[:], g, [[1, g]]), add)

                # ---- leaky relu: out = 0.6*OS + 0.4*|OS| ----
                AB = abpool.tile([128, 1152], f16, tag="ab")
                CC = abpool.tile([128, 1152], f16, tag="cc")
                nc.scalar.activation(AB[:], OS[:],
                                     mybir.ActivationFunctionType.Abs,
                                     scale=0.4)
                nc.scalar.activation(CC[:], OS[:],
                                     mybir.ActivationFunctionType.Copy,
                                     scale=0.6)
                OT = otpool.tile([128, 1152], f16, tag="ot")
                nc.vector.tensor_tensor(OT[:], CC[:], AB[:], add)
                nc.sync.dma_start(out[d], OT[:])

    nc.finalize()
    return nc


def _get_program():
    if "nc" not in _PROG_CACHE:
        _PROG_CACHE["nc"] = _build_program()
    return _PROG_CACHE["nc"]


def _pack_xt(x):  # x [1,32,32,128,128] f32 -> [32, 128, NSLOT*128] fp16
    B, C_, D, H_, W_ = x.shape
    xt = np.zeros((D, 128, NSLOT, 128), np.float32)
    xd = x[0]  # [C, D, H, W]
    s = np.arange(NSLOT)
    m = s % 32
    for t in range(4):
        v = xd[:, :, 4 * m + t, :].reshape(C_, D, NSLOT, 32, 4)  # c d s wb j
        xt[:, 32 * t:32 * t + 32, :, :] = (
            v.transpose(1, 3, 2, 4, 0).reshape(D, 32, NSLOT, 128))
    return np.ascontiguousarray(xt.reshape(D, 128, NSLOT * 128)
                                ).astype(np.float16)


def kernel(x: np.ndarray, y: np.ndarray) -> np.ndarray:
    from concourse.bass_utils import run_bass_kernel_spmd

    x = np.ascontiguousarray(np.asarray(x, dtype=np.float32))
    y = np.ascontiguousarray(np.asarray(y, dtype=np.float32))
    B, C_, D, H_, W_ = x.shape
    assert (B, C_, D, H_, W_) == (1, 32, 32, 128, 128)

    # host prep: depth-shifted, H/W-padded y (fp16); packed XT slabs
    y_sp = np.zeros((D, C_, WPAD, WPAD), np.float16)
    y_sp[1:, :, 1:129, 1:129] = y[0].transpose(1, 0, 2, 3)[:-1].astype(
        np.float16)
    xt = _pack_xt(x)

    nc = _get_program()
    in_maps = [
        {"xtin": xt[4 * j:4 * j + 4], "yin": y_sp[4 * j:4 * j + 4]}
        for j in range(N_CORES)
    ]
    res = run_bass_kernel_spmd(nc, in_maps, core_ids=list(range(N_CORES)),
                               trace=_RUN_OPTS["trace"])
    _LAST_RESULT["res"] = res
    packed = np.concatenate(
        [np.asarray(res.results[j]["out"], np.float32)
         for j in range(N_CORES)], axis=0)  # [32, 128, 1152]

    # host unpermute: [d, p, c'*36 + k*4 + j] -> [1, 9, D, H, W]
    a = packed.reshape(D, 4, 32, 32, 9, 4)                 # d t wb c' k j
    a = a.transpose(3, 4, 0, 1, 2, 5)                      # c' k d t wb j
    a = np.ascontiguousarray(a).reshape(9, 32, D, 4, 32, 4)  # k2 m d t wb j
    a = a.transpose(0, 2, 1, 3, 4, 5)                      # k2 d m t wb j
    a = np.ascontiguousarray(a).reshape(9, D, 128, 128)
    return a[None].astype(np.float32)


# revision 8
# speedup vs baseline: 1.3136x; 1.0959x over previous
"""Trainium2 Bass kernel for nn_CorrTorch_unfold (B=1, C=32, D=32, H=W=128).

Math (flat-remap unfold, see reference docstring): per depth slice d
  out[k2, h2, w2] = lrelu( sum_c x[c,d,h2,w2] * y_pad[c', d, h'+kh, w'+kw+c] )
  with n = 9c'+k' (k'=(kh,kw)), (k2, m) = divmod(n, 32),
  h2 = 4m+t, w2 = 4wb+j, partition p = h' = 32t+wb.

v2 design (all fp16, everything on DVE at the 2x_1p rate):
  - GpSimd (Pool) is NOT used: concurrent Pool tensor_tensor degrades DVE
    ~2-4x via the shared SBUF port pair (measured), a large net loss.
  - A[p, kh*4160 + c'*130 + w] = y_pad[c', d, p+kh, w]   (3 row-shifted DMAs)
  - XT64[p, s*128 + j*32 + c] = x[c, d, 4*(s%32)+t, 4wb+j], 64 m-slots
    (two copies of the 32 m-slots) so each 4-unit product group reads a
    fully contiguous 4608-elem slab: group k covers slots 4k .. 4k+35.
  - products: per group of 4 c'-units one TT mult [128, 4608]
    (in0 A 4-dim strided AP, in1/out contiguous, all even offsets -> 2x).
  - tree: chunks of 16 units (g=576), strided pairwise adds, all even
    offsets (2x); last level via "plane-split" T4 (1x write) so the final
    add reads two contiguous planes instead of an odd-offset stride-2 AP.
  - lrelu = 0.6*OS + 0.4*|OS|: two ACT passes + one DVE add (ACT has its
    own SBUF ports; never contends).

Sharding: D=32 depth slices, 4 per core across 8 cores. Host packs/unpacks
(pure permutations); device output is OS-packed [d, 128, 1152] fp16.
"""
import numpy as np

_PROG_CACHE = {}
_RUN_OPTS = {"trace": False}
_LAST_RESULT = {}

D_LOC = 4
N_CORES = 8
C = 32
H = W = 128
WPAD = 130
ABLK = C * WPAD          # 4160 elems per kh block of A
NSLOT = 64               # XT m-slots (two copies of 32)


def _build_program():
    import concourse.bass as bass
    import concourse.bacc as bacc
    import concourse.mybir as mybir
    from concourse.tile import TileContext
    from bass_rust import VecI64Pair

    f16 = mybir.dt.float16
    mult = mybir.AluOpType.mult
    add = mybir.AluOpType.add

    def apv(base_ap, offset, dims):
        a = base_ap.copy()
        part = list(a.ap[0])
        a.ap = VecI64Pair([part] + [list(d) for d in dims])
        a.offset = a.offset + offset
        return a

    nc = bacc.Bacc()
    xt_in = nc.dram_tensor("xtin", [D_LOC, 128, NSLOT * 128], f16,
                           kind="ExternalInput")
    y_in = nc.dram_tensor("yin", [D_LOC, C, WPAD, WPAD], f16,
                          kind="ExternalInput")
    out = nc.dram_tensor("out", [D_LOC, 128, 1152], f16,
                         kind="ExternalOutput")

    with TileContext(nc) as tc:
        with tc.tile_pool(name="a", bufs=2) as apool, \
             tc.tile_pool(name="xt", bufs=2) as xtpool, \
             tc.tile_pool(name="pr", bufs=1) as prpool, \
             tc.tile_pool(name="tr", bufs=1) as trpool, \
             tc.tile_pool(name="os", bufs=2) as ospool, \
             tc.tile_pool(name="ab", bufs=1) as abpool, \
             tc.tile_pool(name="ot", bufs=2) as otpool:

            for d in range(D_LOC):
                # ---- loads ----
                A = apool.tile([128, 3 * ABLK], f16)
                for kh in range(3):
                    src = y_in[:].copy()
                    src.ap = VecI64Pair(
                        [[WPAD, 128], [WPAD * WPAD, C], [1, WPAD]])
                    src.offset = d * C * WPAD * WPAD + kh * WPAD
                    dst = A[:, kh * ABLK:(kh + 1) * ABLK].rearrange(
                        "p (c w) -> p c w", c=C)
                    nc.sync.dma_start(dst, src)

                XT = xtpool.tile([128, NSLOT * 128], f16)
                nc.sync.dma_start(XT[:], xt_in[d])

                OS = ospool.tile([128, 1152], f16)

                # ---- 2 chunks of 16 c'-units ----
                for ch in range(2):
                    c0 = 16 * ch
                    P = prpool.tile([128, 16 * 1152], f16, tag="p")
                    # products: 4 groups of 4 units x 3 kh, one TT each
                    # (TensorTensor ISA mem patterns allow at most 3 free
                    # dims, so the kh axis gets its own instruction)
                    for gi in range(4):
                        kk = 4 * ch + gi            # global group id 0..7
                        cp0 = 4 * kk                # first c' of group
                        s0 = 4 * kk                 # first XT slot of group
                        for kh in range(3):
                            in0 = apv(A[:], kh * ABLK + cp0 * WPAD,
                                      [[WPAD, 4], [1, 3], [1, 128]])
                            in1 = apv(XT[:], (s0 + 3 * kh) * 128,
                                      [[1152, 4], [1, 384]])
                            po = apv(P[:], gi * 4608 + kh * 384,
                                     [[1152, 4], [1, 384]])
                            nc.vector.tensor_tensor(po, in0, in1, mult)
                    # tree over c2 (innermost 32), g = 16*36 = 576 groups
                    g = 576
                    T1 = trpool.tile([128, g * 16], f16, tag="t1")
                    nc.vector.tensor_tensor(
                        apv(T1[:], 0, [[1, g * 16]]),
                        apv(P[:], 0, [[32, g], [1, 16]]),
                        apv(P[:], 16, [[32, g], [1, 16]]), add)
                    T2 = trpool.tile([128, g * 8], f16, tag="t2")
                    nc.vector.tensor_tensor(
                        apv(T2[:], 0, [[1, g * 8]]),
                        apv(T1[:], 0, [[16, g], [1, 8]]),
                        apv(T1[:], 8, [[16, g], [1, 8]]), add)
                    # T3 sums c2-bit2.  4-elem-run reads measured ~0.44x,
                    # so split into two 2-elem-run instructions, each
                    # writing a contiguous (g, b0) plane for its b1 value.
                    # Tail: one tensor_reduce [g,8] -> [g] (1x, but the
                    # strided-pair TT alternatives measure 0.2-0.8x, and
                    # reduce accumulates in fp32).
                    with nc.allow_low_precision(reason="fp16 out, matches"
                                                " fp16 pairwise tree"):
                        nc.vector.tensor_reduce(
                            apv(OS[:], c0 * 36, [[1, g]]),
                            apv(T2[:], 0, [[8, g], [1, 8]]),
                            mybir.AxisListType.X, add)

                # ---- leaky relu: out = 0.6*OS + 0.4*|OS| ----
                AB = abpool.tile([128, 1152], f16, tag="ab")
                CC = abpool.tile([128, 1152], f16, tag="cc")
                nc.scalar.activation(AB[:], OS[:],
                                     mybir.ActivationFunctionType.Abs,
                                     scale=0.4)
                nc.scalar.activation(CC[:], OS[:],
                                     mybir.ActivationFunctionType.Copy,
                                     scale=0.6)
                OT = otpool.tile([128, 1152], f16, tag="ot")
                nc.vector.tensor_tensor(OT[:], CC[:], AB[:], add)
                nc.sync.dma_start(out[d], OT[:])

    nc.finalize()
    return nc


def _get_program():
    if "nc" not in _PROG_CACHE:
        _PROG_CACHE["nc"] = _build_program()
    return _PROG_CACHE["nc"]


def _pack_xt(x):  # x [1,32,32,128,128] f32 -> [32, 128, NSLOT*128] fp16
    B, C_, D, H_, W_ = x.shape
    xt = np.zeros((D, 128, NSLOT, 128), np.float32)
    xd = x[0]  # [C, D, H, W]
    s = np.arange(NSLOT)
    m = s % 32
    for t in range(4):
        v = xd[:, :, 4 * m + t, :].reshape(C_, D, NSLOT, 32, 4)  # c d s wb j
        xt[:, 32 * t:32 * t + 32, :, :] = (
            v.transpose(1, 3, 2, 4, 0).reshape(D, 32, NSLOT, 128))
    return np.ascontiguousarray(xt.reshape(D, 128, NSLOT * 128)
                                ).astype(np.float16)


def kernel(x: np.ndarray, y: np.ndarray) -> np.ndarray:
    from concourse.bass_utils import run_bass_kernel_spmd

    x = np.ascontiguousarray(np.asarray(x, dtype=np.float32))
    y = np.ascontiguousarray(np.asarray(y, dtype=np.float32))
    B, C_, D, H_, W_ = x.shape
    assert (B, C_, D, H_, W_) == (1, 32, 32, 128, 128)

    # host prep: depth-shifted, H/W-padded y (fp16); packed XT slabs
    y_sp = np.zeros((D, C_, WPAD, WPAD), np.float16)
    y_sp[1:, :, 1:129, 1:129] = y[0].transpose(1, 0, 2, 3)[:-1].astype(
        np.float16)
    xt = _pack_xt(x)

    nc = _get_program()
    in_maps = [
        {"xtin": xt[4 * j:4 * j + 4], "yin": y_sp[4 * j:4 * j + 4]}
        for j in range(N_CORES)
    ]
    res = run_bass_kernel_spmd(nc, in_maps, core_ids=list(range(N_CORES)),
                               trace=_RUN_OPTS["trace"])
    _LAST_RESULT["res"] = res
    packed = np.concatenate(
        [np.asarray(res.results[j]["out"], np.float32)
         for j in range(N_CORES)], axis=0)  # [32, 128, 1152]

    # host unpermute: [d, p, c'*36 + k*4 + j] -> [1, 9, D, H, W]
    a = packed.reshape(D, 4, 32, 32, 9, 4)                 # d t wb c' k j
    a = a.transpose(3, 4, 0, 1, 2, 5)                      # c' k d t wb j
    a = np.ascontiguousarray(a).reshape(9, 32, D, 4, 32, 4)  # k2 m d t wb j
    a = a.transpose(0, 2, 1, 3, 4, 5)                      # k2 d m t wb j
    a = np.ascontiguousarray(a).reshape(9, D, 128, 128)
    return a[None].astype(np.float32)


# revision 9
# speedup vs baseline: 1.3684x; 1.0417x over previous
"""Trainium2 Bass kernel for nn_CorrTorch_unfold (B=1, C=32, D=32, H=W=128).

Math (flat-remap unfold, see reference docstring): per depth slice d
  out[k2, h2, w2] = lrelu( sum_c x[c,d,h2,w2] * y_pad[c', d, h'+kh, w'+kw+c] )
  with n = 9c'+k' (k'=(kh,kw)), (k2, m) = divmod(n, 32),
  h2 = 4m+t, w2 = 4wb+j, partition p = h' = 32t+wb.

v2 design (all fp16, everything on DVE at the 2x_1p rate):
  - GpSimd (Pool) is NOT used: concurrent Pool tensor_tensor degrades DVE
    ~2-4x via the shared SBUF port pair (measured), a large net loss.
  - A[p, kh*4160 + c'*130 + w] = y_pad[c', d, p+kh, w]   (3 row-shifted DMAs)
  - XT64[p, s*128 + j*32 + c] = x[c, d, 4*(s%32)+t, 4wb+j], 64 m-slots
    (two copies of the 32 m-slots) so each 4-unit product group reads a
    fully contiguous 4608-elem slab: group k covers slots 4k .. 4k+35.
  - products: per group of 4 c'-units one TT mult [128, 4608]
    (in0 A 4-dim strided AP, in1/out contiguous, all even offsets -> 2x).
  - tree: chunks of 16 units (g=576), strided pairwise adds, all even
    offsets (2x); last level via "plane-split" T4 (1x write) so the final
    add reads two contiguous planes instead of an odd-offset stride-2 AP.
  - lrelu = 0.6*OS + 0.4*|OS|: two ACT passes + one DVE add (ACT has its
    own SBUF ports; never contends).

Sharding: D=32 depth slices, 4 per core across 8 cores. Host packs/unpacks
(pure permutations); device output is OS-packed [d, 128, 1152] fp16.
"""
import numpy as np

_PROG_CACHE = {}
_RUN_OPTS = {"trace": False}
_LAST_RESULT = {}

D_LOC = 4
N_CORES = 8
C = 32
H = W = 128
WPAD = 130
ABLK = C * WPAD          # 4160 elems per kh block of A
NSLOT = 64               # XT m-slots (two copies of 32)


def _build_program():
    import concourse.bass as bass
    import concourse.bacc as bacc
    import concourse.mybir as mybir
    from concourse.tile import TileContext
    from bass_rust import VecI64Pair

    f16 = mybir.dt.float16
    mult = mybir.AluOpType.mult
    add = mybir.AluOpType.add

    def apv(base_ap, offset, dims):
        a = base_ap.copy()
        part = list(a.ap[0])
        a.ap = VecI64Pair([part] + [list(d) for d in dims])
        a.offset = a.offset + offset
        return a

    nc = bacc.Bacc()
    xt_in = nc.dram_tensor("xtin", [D_LOC, 128, NSLOT * 128], f16,
                           kind="ExternalInput")
    y_in = nc.dram_tensor("yin", [D_LOC, WPAD, C, WPAD], f16,
                          kind="ExternalInput")
    out = nc.dram_tensor("out", [D_LOC, 128, 1152], f16,
                         kind="ExternalOutput")

    with TileContext(nc) as tc:
        with tc.tile_pool(name="a", bufs=2) as apool, \
             tc.tile_pool(name="xt", bufs=2) as xtpool, \
             tc.tile_pool(name="pr", bufs=1) as prpool, \
             tc.tile_pool(name="tr", bufs=1) as trpool, \
             tc.tile_pool(name="os", bufs=2) as ospool, \
             tc.tile_pool(name="ab", bufs=1) as abpool, \
             tc.tile_pool(name="ot", bufs=2) as otpool:

            for d in range(D_LOC):
                # ---- loads ----
                A = apool.tile([128, 3 * ABLK], f16)
                # one DMA: partition p reads rows p, p+1, p+2 of
                # y_sp2[d] = [WPAD rows, C*WPAD contiguous each]
                src = y_in[:].copy()
                src.ap = VecI64Pair(
                    [[ABLK, 128], [ABLK, 3], [1, ABLK]])
                src.offset = d * WPAD * ABLK
                dst = A[:].copy()
                dst.ap = VecI64Pair([list(dst.ap[0]), [1, 3 * ABLK]])
                nc.sync.dma_start(dst, src)

                XT = xtpool.tile([128, NSLOT * 128], f16)
                nc.sync.dma_start(XT[:], xt_in[d])

                OS = ospool.tile([128, 1152], f16)

                # ---- 2 chunks of 16 c'-units ----
                for ch in range(2):
                    c0 = 16 * ch
                    P = prpool.tile([128, 16 * 1152], f16, tag="p")
                    # products: 4 groups of 4 units x 3 kh, one TT each
                    # (TensorTensor ISA mem patterns allow at most 3 free
                    # dims, so the kh axis gets its own instruction)
                    for gi in range(4):
                        kk = 4 * ch + gi            # global group id 0..7
                        cp0 = 4 * kk                # first c' of group
                        s0 = 4 * kk                 # first XT slot of group
                        for kh in range(3):
                            in0 = apv(A[:], kh * ABLK + cp0 * WPAD,
                                      [[WPAD, 4], [1, 3], [1, 128]])
                            in1 = apv(XT[:], (s0 + 3 * kh) * 128,
                                      [[1152, 4], [1, 384]])
                            po = apv(P[:], gi * 4608 + kh * 384,
                                     [[1152, 4], [1, 384]])
                            nc.vector.tensor_tensor(po, in0, in1, mult)
                    # tree over c2 (innermost 32), g = 16*36 = 576 groups
                    g = 576
                    T1 = trpool.tile([128, g * 16], f16, tag="t1")
                    nc.vector.tensor_tensor(
                        apv(T1[:], 0, [[1, g * 16]]),
                        apv(P[:], 0, [[32, g], [1, 16]]),
                        apv(P[:], 16, [[32, g], [1, 16]]), add)
                    T2 = trpool.tile([128, g * 8], f16, tag="t2")
                    nc.vector.tensor_tensor(
                        apv(T2[:], 0, [[1, g * 8]]),
                        apv(T1[:], 0, [[16, g], [1, 8]]),
                        apv(T1[:], 8, [[16, g], [1, 8]]), add)
                    # T3 sums c2-bit2.  4-elem-run reads measured ~0.44x,
                    # so split into two 2-elem-run instructions, each
                    # writing a contiguous (g, b0) plane for its b1 value.
                    # Tail: T3 sums bit2 via two 2-elem-run TTs into a
                    # quad layout (both measured ~1.09us), then one
                    # contiguous tensor_reduce [g,4] -> [g] (fp32 accum).
                    T3 = trpool.tile([128, g * 4], f16, tag="t3")
                    nc.vector.tensor_tensor(
                        apv(T3[:], 0, [[4, g], [1, 2]]),
                        apv(T2[:], 0, [[8, g], [1, 2]]),
                        apv(T2[:], 4, [[8, g], [1, 2]]), add)
                    nc.vector.tensor_tensor(
                        apv(T3[:], 2, [[4, g], [1, 2]]),
                        apv(T2[:], 2, [[8, g], [1, 2]]),
                        apv(T2[:], 6, [[8, g], [1, 2]]), add)
                    with nc.allow_low_precision(reason="fp16 out, matches"
                                                " fp16 pairwise tree"):
                        nc.vector.tensor_reduce(
                            apv(OS[:], c0 * 36, [[1, g]]),
                            apv(T3[:], 0, [[4, g], [1, 4]]),
                            mybir.AxisListType.X, add)

                # ---- leaky relu: out = 0.6*OS + 0.4*|OS| ----
                AB = abpool.tile([128, 1152], f16, tag="ab")
                CC = abpool.tile([128, 1152], f16, tag="cc")
                nc.scalar.activation(AB[:], OS[:],
                                     mybir.ActivationFunctionType.Abs,
                                     scale=0.4)
                nc.scalar.activation(CC[:], OS[:],
                                     mybir.ActivationFunctionType.Copy,
                                     scale=0.6)
                OT = otpool.tile([128, 1152], f16, tag="ot")
                nc.vector.tensor_tensor(OT[:], CC[:], AB[:], add)
                nc.sync.dma_start(out[d], OT[:])

    nc.finalize()
    return nc


def _get_program():
    if "nc" not in _PROG_CACHE:
        _PROG_CACHE["nc"] = _build_program()
    return _PROG_CACHE["nc"]


def _pack_xt(x):  # x [1,32,32,128,128] f32 -> [32, 128, NSLOT*128] fp16
    B, C_, D, H_, W_ = x.shape
    xt = np.zeros((D, 128, NSLOT, 128), np.float32)
    xd = x[0]  # [C, D, H, W]
    s = np.arange(NSLOT)
    m = s % 32
    for t in range(4):
        v = xd[:, :, 4 * m + t, :].reshape(C_, D, NSLOT, 32, 4)  # c d s wb j
        xt[:, 32 * t:32 * t + 32, :, :] = (
            v.transpose(1, 3, 2, 4, 0).reshape(D, 32, NSLOT, 128))
    return np.ascontiguousarray(xt.reshape(D, 128, NSLOT * 128)
                                ).astype(np.float16)


def kernel(x: np.ndarray, y: np.ndarray) -> np.ndarray:
    from concourse.bass_utils import run_bass_kernel_spmd

    x = np.ascontiguousarray(np.asarray(x, dtype=np.float32))
    y = np.ascontiguousarray(np.asarray(y, dtype=np.float32))
    B, C_, D, H_, W_ = x.shape
    assert (B, C_, D, H_, W_) == (1, 32, 32, 128, 128)

    # host prep: depth-shifted, H/W-padded y (fp16); packed XT slabs
    y_sp = np.zeros((D, WPAD, C_, WPAD), np.float16)
    y_sp[1:, 1:129, :, 1:129] = y[0].transpose(1, 2, 0, 3)[:-1].astype(
        np.float16)
    xt = _pack_xt(x)

    nc = _get_program()
    in_maps = [
        {"xtin": xt[4 * j:4 * j + 4], "yin": y_sp[4 * j:4 * j + 4]}
        for j in range(N_CORES)
    ]
    res = run_bass_kernel_spmd(nc, in_maps, core_ids=list(range(N_CORES)),
                               trace=_RUN_OPTS["trace"])
    _LAST_RESULT["res"] = res
    packed = np.concatenate(
        [np.asarray(res.results[j]["out"], np.float32)
         for j in range(N_CORES)], axis=0)  # [32, 128, 1152]

    # host unpermute: [d, p, c'*36 + k*4 + j] -> [1, 9, D, H, W]
    a = packed.reshape(D, 4, 32, 32, 9, 4)                 # d t wb c' k j
    a = a.transpose(3, 4, 0, 1, 2, 5)                      # c' k d t wb j
    a = np.ascontiguousarray(a).reshape(9, 32, D, 4, 32, 4)  # k2 m d t wb j
    a = a.transpose(0, 2, 1, 3, 4, 5)                      # k2 d m t wb j
    a = np.ascontiguousarray(a).reshape(9, D, 128, 128)
    return a[None].astype(np.float32)


# revision 10
# speedup vs baseline: 1.3853x; 1.0124x over previous
"""Trainium2 Bass kernel for nn_CorrTorch_unfold (B=1, C=32, D=32, H=W=128).

Math (flat-remap unfold, see reference docstring): per depth slice d
  out[k2, h2, w2] = lrelu( sum_c x[c,d,h2,w2] * y_pad[c', d, h'+kh, w'+kw+c] )
  with n = 9c'+k' (k'=(kh,kw)), (k2, m) = divmod(n, 32),
  h2 = 4m+t, w2 = 4wb+j, partition p = h' = 32t+wb.

v2 design (all fp16, everything on DVE at the 2x_1p rate):
  - GpSimd (Pool) is NOT used: concurrent Pool tensor_tensor degrades DVE
    ~2-4x via the shared SBUF port pair (measured), a large net loss.
  - A[p, kh*4160 + c'*130 + w] = y_pad[c', d, p+kh, w]   (3 row-shifted DMAs)
  - XT64[p, s*128 + j*32 + c] = x[c, d, 4*(s%32)+t, 4wb+j], 64 m-slots
    (two copies of the 32 m-slots) so each 4-unit product group reads a
    fully contiguous 4608-elem slab: group k covers slots 4k .. 4k+35.
  - products: per group of 4 c'-units one TT mult [128, 4608]
    (in0 A 4-dim strided AP, in1/out contiguous, all even offsets -> 2x).
  - tree: chunks of 16 units (g=576), strided pairwise adds, all even
    offsets (2x); last level via "plane-split" T4 (1x write) so the final
    add reads two contiguous planes instead of an odd-offset stride-2 AP.
  - lrelu = 0.6*OS + 0.4*|OS|: two ACT passes + one DVE add (ACT has its
    own SBUF ports; never contends).

Sharding: D=32 depth slices, 4 per core across 8 cores. Host packs/unpacks
(pure permutations); device output is OS-packed [d, 128, 1152] fp16.
"""
import numpy as np

_PROG_CACHE = {}
_RUN_OPTS = {"trace": False}
_LAST_RESULT = {}

D_LOC = 4
N_CORES = 8
C = 32
H = W = 128
WPAD = 130
ABLK = C * WPAD          # 4160 elems per kh block of A
NSLOT = 64               # XT m-slots (two copies of 32)


def _build_program():
    import concourse.bass as bass
    import concourse.bacc as bacc
    import concourse.mybir as mybir
    from concourse.tile import TileContext
    from bass_rust import VecI64Pair

    f16 = mybir.dt.float16
    mult = mybir.AluOpType.mult
    add = mybir.AluOpType.add

    def apv(base_ap, offset, dims):
        a = base_ap.copy()
        part = list(a.ap[0])
        a.ap = VecI64Pair([part] + [list(d) for d in dims])
        a.offset = a.offset + offset
        return a

    nc = bacc.Bacc()
    xt_in = nc.dram_tensor("xtin", [D_LOC, 128, NSLOT * 128], f16,
                           kind="ExternalInput")
    y_in = nc.dram_tensor("yin", [D_LOC, WPAD, C, WPAD], f16,
                          kind="ExternalInput")
    out = nc.dram_tensor("out", [D_LOC, 128, 1152], f16,
                         kind="ExternalOutput")

    with TileContext(nc) as tc:
        with tc.tile_pool(name="a", bufs=2) as apool, \
             tc.tile_pool(name="xt", bufs=2) as xtpool, \
             tc.tile_pool(name="pr", bufs=1) as prpool, \
             tc.tile_pool(name="tr", bufs=1) as trpool, \
             tc.tile_pool(name="os", bufs=2) as ospool, \
             tc.tile_pool(name="ab", bufs=1) as abpool, \
             tc.tile_pool(name="ot", bufs=2) as otpool:

            for d in range(D_LOC):
                # ---- loads ----
                A = apool.tile([128, 3 * ABLK], f16)
                # per-kh DMA: partition p reads row p+kh of
                # y_sp2[d] = [WPAD rows, C*WPAD contiguous each];
                # split so kh=0 products can start before kh=1/2 land.
                for kh in range(3):
                    src = y_in[:].copy()
                    src.ap = VecI64Pair([[ABLK, 128], [1, ABLK]])
                    src.offset = (d * WPAD + kh) * ABLK
                    nc.sync.dma_start(A[:, kh * ABLK:(kh + 1) * ABLK], src)

                XT = xtpool.tile([128, NSLOT * 128], f16)
                nc.sync.dma_start(XT[:, :48 * 128], xt_in[d, :, :48 * 128])
                nc.sync.dma_start(XT[:, 48 * 128:], xt_in[d, :, 48 * 128:])

                OS = ospool.tile([128, 1152], f16)

                # ---- 2 chunks of 16 c'-units ----
                for ch in range(2):
                    c0 = 16 * ch
                    P = prpool.tile([128, 16 * 1152], f16, tag="p")
                    # products: 4 groups of 4 units x 3 kh, one TT each
                    # (TensorTensor ISA mem patterns allow at most 3 free
                    # dims, so the kh axis gets its own instruction)
                    for kh in range(3):
                      for gi in range(4):
                        kk = 4 * ch + gi            # global group id 0..7
                        cp0 = 4 * kk                # first c' of group
                        s0 = 4 * kk                 # first XT slot of group
                        if True:
                            in0 = apv(A[:], kh * ABLK + cp0 * WPAD,
                                      [[WPAD, 4], [1, 3], [1, 128]])
                            in1 = apv(XT[:], (s0 + 3 * kh) * 128,
                                      [[1152, 4], [1, 384]])
                            po = apv(P[:], gi * 4608 + kh * 384,
                                     [[1152, 4], [1, 384]])
                            nc.vector.tensor_tensor(po, in0, in1, mult)
                    # tree over c2 (innermost 32), g = 16*36 = 576 groups
                    g = 576
                    T1 = trpool.tile([128, g * 16], f16, tag="t1")
                    nc.vector.tensor_tensor(
                        apv(T1[:], 0, [[1, g * 16]]),
                        apv(P[:], 0, [[32, g], [1, 16]]),
                        apv(P[:], 16, [[32, g], [1, 16]]), add)
                    T2 = trpool.tile([128, g * 8], f16, tag="t2")
                    nc.vector.tensor_tensor(
                        apv(T2[:], 0, [[1, g * 8]]),
                        apv(T1[:], 0, [[16, g], [1, 8]]),
                        apv(T1[:], 8, [[16, g], [1, 8]]), add)
                    # T3 sums c2-bit2.  4-elem-run reads measured ~0.44x,
                    # so split into two 2-elem-run instructions, each
                    # writing a contiguous (g, b0) plane for its b1 value.
                    # Tail: T3 sums bit2 via two 2-elem-run TTs into a
                    # quad layout (both measured ~1.09us), then one
                    # contiguous tensor_reduce [g,4] -> [g] (fp32 accum).
                    T3 = trpool.tile([128, g * 4], f16, tag="t3")
                    nc.vector.tensor_tensor(
                        apv(T3[:], 0, [[4, g], [1, 2]]),
                        apv(T2[:], 0, [[8, g], [1, 2]]),
                        apv(T2[:], 4, [[8, g], [1, 2]]), add)
                    nc.vector.tensor_tensor(
                        apv(T3[:], 2, [[4, g], [1, 2]]),
                        apv(T2[:], 2, [[8, g], [1, 2]]),
                        apv(T2[:], 6, [[8, g], [1, 2]]), add)
                    with nc.allow_low_precision(reason="fp16 out, matches"
                                                " fp16 pairwise tree"):
                        nc.vector.tensor_reduce(
                            apv(OS[:], c0 * 36, [[1, g]]),
                            apv(T3[:], 0, [[4, g], [1, 4]]),
                            mybir.AxisListType.X, add)

                # ---- leaky relu: out = 0.6*OS + 0.4*|OS| ----
                AB = abpool.tile([128, 1152], f16, tag="ab")
                CC = abpool.tile([128, 1152], f16, tag="cc")
                nc.scalar.activation(AB[:], OS[:],
                                     mybir.ActivationFunctionType.Abs,
                                     scale=0.4)
                nc.scalar.activation(CC[:], OS[:],
                                     mybir.ActivationFunctionType.Copy,
                                     scale=0.6)
                OT = otpool.tile([128, 1152], f16, tag="ot")
                nc.vector.tensor_tensor(OT[:], CC[:], AB[:], add)
                nc.sync.dma_start(out[d], OT[:])

    nc.finalize()
    return nc


def _get_program():
    if "nc" not in _PROG_CACHE:
        _PROG_CACHE["nc"] = _build_program()
    return _PROG_CACHE["nc"]


def _pack_xt(x):  # x [1,32,32,128,128] f32 -> [32, 128, NSLOT*128] fp16
    B, C_, D, H_, W_ = x.shape
    xt = np.zeros((D, 128, NSLOT, 128), np.float32)
    xd = x[0]  # [C, D, H, W]
    s = np.arange(NSLOT)
    m = s % 32
    for t in range(4):
        v = xd[:, :, 4 * m + t, :].reshape(C_, D, NSLOT, 32, 4)  # c d s wb j
        xt[:, 32 * t:32 * t + 32, :, :] = (
            v.transpose(1, 3, 2, 4, 0).reshape(D, 32, NSLOT, 128))
    return np.ascontiguousarray(xt.reshape(D, 128, NSLOT * 128)
                                ).astype(np.float16)


def kernel(x: np.ndarray, y: np.ndarray) -> np.ndarray:
    from concourse.bass_utils import run_bass_kernel_spmd

    x = np.ascontiguousarray(np.asarray(x, dtype=np.float32))
    y = np.ascontiguousarray(np.asarray(y, dtype=np.float32))
    B, C_, D, H_, W_ = x.shape
    assert (B, C_, D, H_, W_) == (1, 32, 32, 128, 128)

    # host prep: depth-shifted, H/W-padded y (fp16); packed XT slabs
    y_sp = np.zeros((D, WPAD, C_, WPAD), np.float16)
    y_sp[1:, 1:129, :, 1:129] = y[0].transpose(1, 2, 0, 3)[:-1].astype(
        np.float16)
    xt = _pack_xt(x)

    nc = _get_program()
    in_maps = [
        {"xtin": xt[4 * j:4 * j + 4], "yin": y_sp[4 * j:4 * j + 4]}
        for j in range(N_CORES)
    ]
    res = run_bass_kernel_spmd(nc, in_maps, core_ids=list(range(N_CORES)),
                               trace=_RUN_OPTS["trace"])
    _LAST_RESULT["res"] = res
    packed = np.concatenate(
        [np.asarray(res.results[j]["out"], np.float32)
         for j in range(N_CORES)], axis=0)  # [32, 128, 1152]

    # host unpermute: [d, p, c'*36 + k*4 + j] -> [1, 9, D, H, W]
    a = packed.reshape(D, 4, 32, 32, 9, 4)                 # d t wb c' k j
    a = a.transpose(3, 4, 0, 1, 2, 5)                      # c' k d t wb j
    a = np.ascontiguousarray(a).reshape(9, 32, D, 4, 32, 4)  # k2 m d t wb j
    a = a.transpose(0, 2, 1, 3, 4, 5)                      # k2 d m t wb j
    a = np.ascontiguousarray(a).reshape(9, D, 128, 128)
    return a[None].astype(np.float32)


# revision 11
# speedup vs baseline: 1.4141x; 1.0207x over previous
"""Trainium2 Bass kernel for nn_CorrTorch_unfold (B=1, C=32, D=32, H=W=128).

Math (flat-remap unfold, see reference docstring): per depth slice d
  out[k2, h2, w2] = lrelu( sum_c x[c,d,h2,w2] * y_pad[c', d, h'+kh, w'+kw+c] )
  with n = 9c'+k' (k'=(kh,kw)), (k2, m) = divmod(n, 32),
  h2 = 4m+t, w2 = 4wb+j, partition p = h' = 32t+wb.

v2 design (all fp16, everything on DVE at the 2x_1p rate):
  - GpSimd (Pool) is NOT used: concurrent Pool tensor_tensor degrades DVE
    ~2-4x via the shared SBUF port pair (measured), a large net loss.
  - A[p, kh*4160 + c'*130 + w] = y_pad[c', d, p+kh, w]   (3 row-shifted DMAs)
  - XT64[p, s*128 + j*32 + c] = x[c, d, 4*(s%32)+t, 4wb+j], 64 m-slots
    (two copies of the 32 m-slots) so each 4-unit product group reads a
    fully contiguous 4608-elem slab: group k covers slots 4k .. 4k+35.
  - products: per group of 4 c'-units one TT mult [128, 4608]
    (in0 A 4-dim strided AP, in1/out contiguous, all even offsets -> 2x).
  - tree: chunks of 16 units (g=576), strided pairwise adds, all even
    offsets (2x); last level via "plane-split" T4 (1x write) so the final
    add reads two contiguous planes instead of an odd-offset stride-2 AP.
  - lrelu = 0.6*OS + 0.4*|OS|: two ACT passes + one DVE add (ACT has its
    own SBUF ports; never contends).

Sharding: D=32 depth slices, 4 per core across 8 cores. Host packs/unpacks
(pure permutations); device output is OS-packed [d, 128, 1152] fp16.
"""
import numpy as np

_PROG_CACHE = {}
_RUN_OPTS = {"trace": False}
_LAST_RESULT = {}

D_LOC = 4
N_CORES = 8
C = 32
H = W = 128
WPAD = 130
ABLK = C * WPAD          # 4160 elems per kh block of A
NSLOT = 64               # XT m-slots (two copies of 32)


def _build_program():
    import concourse.bass as bass
    import concourse.bacc as bacc
    import concourse.mybir as mybir
    from concourse.tile import TileContext
    from bass_rust import VecI64Pair

    f16 = mybir.dt.float16
    mult = mybir.AluOpType.mult
    add = mybir.AluOpType.add

    def apv(base_ap, offset, dims):
        a = base_ap.copy()
        part = list(a.ap[0])
        a.ap = VecI64Pair([part] + [list(d) for d in dims])
        a.offset = a.offset + offset
        return a

    nc = bacc.Bacc()
    xt_in = nc.dram_tensor("xtin", [D_LOC, 128, NSLOT * 128], f16,
                           kind="ExternalInput")
    y_in = nc.dram_tensor("yin", [D_LOC, WPAD, C, WPAD], f16,
                          kind="ExternalInput")
    out = nc.dram_tensor("out", [D_LOC, 128, 1152], f16,
                         kind="ExternalOutput")

    with TileContext(nc) as tc:
        with tc.tile_pool(name="a", bufs=2) as apool, \
             tc.tile_pool(name="xt", bufs=2) as xtpool, \
             tc.tile_pool(name="pr", bufs=1) as prpool, \
             tc.tile_pool(name="tr", bufs=1) as trpool, \
             tc.tile_pool(name="os", bufs=2) as ospool, \
             tc.tile_pool(name="ab", bufs=1) as abpool, \
             tc.tile_pool(name="ot", bufs=2) as otpool:

            for d in range(D_LOC):
                # ---- loads ----
                # separate tiles per kh block / XT half so the tile
                # dependency tracker lets kh=0 chunk-0 products start as
                # soon as XTa + A0 land (deps are whole-tile).
                # XTa = slots [0,48) for chunk 0; XTb = slots [16,64)
                # (overlapping) for chunk 1.
                XTa = xtpool.tile([128, 48 * 128], f16, tag="xta")
                nc.sync.dma_start(XTa[:], xt_in[d, :, :48 * 128])
                Akh = []
                for kh in range(3):
                    Ak = apool.tile([128, ABLK], f16, tag=f"a{kh}")
                    src = y_in[:].copy()
                    src.ap = VecI64Pair([[ABLK, 128], [1, ABLK]])
                    src.offset = (d * WPAD + kh) * ABLK
                    nc.sync.dma_start(Ak[:], src)
                    Akh.append(Ak)
                XTb = xtpool.tile([128, 48 * 128], f16, tag="xtb")
                nc.sync.dma_start(XTb[:], xt_in[d, :, 16 * 128:])

                OS = ospool.tile([128, 1152], f16)

                # ---- 2 chunks of 16 c'-units ----
                for ch in range(2):
                    c0 = 16 * ch
                    P = prpool.tile([128, 16 * 1152], f16, tag="p")
                    # products: 4 groups of 4 units x 3 kh, one TT each
                    # (TensorTensor ISA mem patterns allow at most 3 free
                    # dims, so the kh axis gets its own instruction)
                    XTc = XTa if ch == 0 else XTb
                    sbase = 0 if ch == 0 else 16
                    for kh in range(3):
                        for gi in range(4):
                            kk = 4 * ch + gi        # global group id 0..7
                            cp0 = 4 * kk            # first c' of group
                            s0 = 4 * kk             # first XT slot of group
                            in0 = apv(Akh[kh][:], cp0 * WPAD,
                                      [[WPAD, 4], [1, 3], [1, 128]])
                            in1 = apv(XTc[:], (s0 - sbase + 3 * kh) * 128,
                                      [[1152, 4], [1, 384]])
                            po = apv(P[:], gi * 4608 + kh * 384,
                                     [[1152, 4], [1, 384]])
                            nc.vector.tensor_tensor(po, in0, in1, mult)
                    # tree over c2 (innermost 32), g = 16*36 = 576 groups
                    g = 576
                    T1 = trpool.tile([128, g * 16], f16, tag="t1")
                    nc.vector.tensor_tensor(
                        apv(T1[:], 0, [[1, g * 16]]),
                        apv(P[:], 0, [[32, g], [1, 16]]),
                        apv(P[:], 16, [[32, g], [1, 16]]), add)
                    T2 = trpool.tile([128, g * 8], f16, tag="t2")
                    nc.vector.tensor_tensor(
                        apv(T2[:], 0, [[1, g * 8]]),
                        apv(T1[:], 0, [[16, g], [1, 8]]),
                        apv(T1[:], 8, [[16, g], [1, 8]]), add)
                    # T3 sums c2-bit2.  4-elem-run reads measured ~0.44x,
                    # so split into two 2-elem-run instructions, each
                    # writing a contiguous (g, b0) plane for its b1 value.
                    # Tail: T3 sums bit2 via two 2-elem-run TTs into a
                    # quad layout (both measured ~1.09us), then one
                    # contiguous tensor_reduce [g,4] -> [g] (fp32 accum).
                    T3 = trpool.tile([128, g * 4], f16, tag="t3")
                    nc.vector.tensor_tensor(
                        apv(T3[:], 0, [[4, g], [1, 2]]),
                        apv(T2[:], 0, [[8, g], [1, 2]]),
                        apv(T2[:], 4, [[8, g], [1, 2]]), add)
                    nc.vector.tensor_tensor(
                        apv(T3[:], 2, [[4, g], [1, 2]]),
                        apv(T2[:], 2, [[8, g], [1, 2]]),
                        apv(T2[:], 6, [[8, g], [1, 2]]), add)
                    with nc.allow_low_precision(reason="fp16 out, matches"
                                                " fp16 pairwise tree"):
                        nc.vector.tensor_reduce(
                            apv(OS[:], c0 * 36, [[1, g]]),
                            apv(T3[:], 0, [[4, g], [1, 4]]),
                            mybir.AxisListType.X, add)

                    # leaky relu (per chunk): out = 0.6*OS + 0.4*|OS|
                    AB = abpool.tile([128, g], f16, tag="ab")
                    CC = abpool.tile([128, g], f16, tag="cc")
                    nc.scalar.activation(AB[:], OS[:, c0 * 36:c0 * 36 + g],
                                         mybir.ActivationFunctionType.Abs,
                                         scale=0.4)
                    nc.scalar.activation(CC[:], OS[:, c0 * 36:c0 * 36 + g],
                                         mybir.ActivationFunctionType.Copy,
                                         scale=0.6)
                    OT = otpool.tile([128, g], f16, tag="ot")
                    nc.vector.tensor_tensor(OT[:], CC[:], AB[:], add)
                    nc.sync.dma_start(out[d, :, c0 * 36:c0 * 36 + g], OT[:])

    nc.finalize()
    return nc


def _get_program():
    if "nc" not in _PROG_CACHE:
        _PROG_CACHE["nc"] = _build_program()
    return _PROG_CACHE["nc"]


def _pack_xt(x):  # x [1,32,32,128,128] f32 -> [32, 128, NSLOT*128] fp16
    B, C_, D, H_, W_ = x.shape
    xt = np.zeros((D, 128, NSLOT, 128), np.float32)
    xd = x[0]  # [C, D, H, W]
    s = np.arange(NSLOT)
    m = s % 32
    for t in range(4):
        v = xd[:, :, 4 * m + t, :].reshape(C_, D, NSLOT, 32, 4)  # c d s wb j
        xt[:, 32 * t:32 * t + 32, :, :] = (
            v.transpose(1, 3, 2, 4, 0).reshape(D, 32, NSLOT, 128))
    return np.ascontiguousarray(xt.reshape(D, 128, NSLOT * 128)
                                ).astype(np.float16)


def kernel(x: np.ndarray, y: np.ndarray) -> np.ndarray:
    from concourse.bass_utils import run_bass_kernel_spmd

    x = np.ascontiguousarray(np.asarray(x, dtype=np.float32))
    y = np.ascontiguousarray(np.asarray(y, dtype=np.float32))
    B, C_, D, H_, W_ = x.shape
    assert (B, C_, D, H_, W_) == (1, 32, 32, 128, 128)

    # host prep: depth-shifted, H/W-padded y (fp16); packed XT slabs
    y_sp = np.zeros((D, WPAD, C_, WPAD), np.float16)
    y_sp[1:, 1:129, :, 1:129] = y[0].transpose(1, 2, 0, 3)[:-1].astype(
        np.float16)
    xt = _pack_xt(x)

    nc = _get_program()
    in_maps = [
        {"xtin": xt[4 * j:4 * j + 4], "yin": y_sp[4 * j:4 * j + 4]}
        for j in range(N_CORES)
    ]
    res = run_bass_kernel_spmd(nc, in_maps, core_ids=list(range(N_CORES)),
                               trace=_RUN_OPTS["trace"])
    _LAST_RESULT["res"] = res
    packed = np.concatenate(
        [np.asarray(res.results[j]["out"], np.float32)
         for j in range(N_CORES)], axis=0)  # [32, 128, 1152]

    # host unpermute: [d, p, c'*36 + k*4 + j] -> [1, 9, D, H, W]
    a = packed.reshape(D, 4, 32, 32, 9, 4)                 # d t wb c' k j
    a = a.transpose(3, 4, 0, 1, 2, 5)                      # c' k d t wb j
    a = np.ascontiguousarray(a).reshape(9, 32, D, 4, 32, 4)  # k2 m d t wb j
    a = a.transpose(0, 2, 1, 3, 4, 5)                      # k2 d m t wb j
    a = np.ascontiguousarray(a).reshape(9, D, 128, 128)
    return a[None].astype(np.float32)
